# revision 1
# baseline (speedup 1.0000x reference)
"""Trainium2 Bass kernel for nn_NeighborAttention.

Key algebraic structure exploited: the attention query is a single
mean-pooled neighbor vector per batch, broadcast over the sequence.
Hence the [seq, seq] attention collapses to one weight vector per head
([nh, seq]) and the whole attention output is a single vector per batch
added to every row of x before the final LayerNorm.  The k/v
projections are never materialized: scores are computed as
x @ (q^T kw) and the value path as (w @ x) @ vw^T, reducing compute
from ~34 GFLOP to ~0.6 GFLOP.  Sharding: data-parallel over batch
(one batch element per NeuronCore, 8 cores).

Host-side prep is limited to sharding/layout/dtype: transposed + bf16
copies of weights (perturbative paths only; residual/LN stay fp32).
"""

import numpy as np
import ml_dtypes
from contextlib import ExitStack

try:
    import concourse.bass as bass
except ImportError:  # pragma: no cover - fallback for bare containers
    import sys
    sys.path.insert(0, "/opt/trn_rl_repo")
    import concourse.bass as bass

import concourse.tile as tile
from concourse import bacc, mybir
from concourse import bass_utils
from concourse.alu_op_type import AluOpType

F32 = mybir.dt.float32
BF16 = mybir.dt.bfloat16
I32 = mybir.dt.int32
AF = mybir.ActivationFunctionType
AX = mybir.AxisListType

BS, SEQ, DIM, NH, DH, NNB = 8, 1024, 1024, 16, 64, 50
NT = SEQ // 128   # seq tiles
NJ = DIM // 128   # dim chunks
LN_EPS = 1e-12
N_CORES = 8

_cache = {}


def _build(flags):
    use_qb, use_kb, use_vb, use_ob, use_g, use_b, use_mask = flags
    nc = bacc.Bacc("TRN2", target_bir_lowering=False, debug=False,
                   enable_asserts=True, num_devices=N_CORES)

    def din(name, shape, dt):
        return nc.dram_tensor(name, shape, dt, kind="ExternalInput").ap()

    x_d = din("x", [SEQ, DIM], F32)
    xt_d = din("xt", [DIM, SEQ], BF16)
    qwt_d = din("qwt", [DIM, DIM], BF16)
    kw_d = din("kw", [DIM, DIM], BF16)
    vwt_d = din("vwt", [DIM, DIM], BF16)
    owt_d = din("owt", [DIM, DIM], BF16)
    xbn_d = din("xbn", [SEQ, DIM], BF16)
    xnb_d = din("xnb", [NNB, DIM], F32)
    nm_d = din("nm", [NNB], F32)
    i128_d = din("i128", [128, 128], BF16)
    qb_d = din("qb", [DIM], F32) if use_qb else None
    kbt_d = din("kbt", [128, NJ], BF16) if use_kb else None
    vb_d = din("vbt", [128, NJ], BF16) if use_vb else None
    ob_d = din("ob", [DIM], F32) if use_ob else None
    g_d = din("lng", [DIM], F32) if use_g else None
    b_d = din("lnb", [DIM], F32) if use_b else None
    mask_d = din("mask", [SEQ], I32) if use_mask else None
    out_d = nc.dram_tensor("out", [SEQ, DIM], F32, kind="ExternalOutput").ap()

    with tile.TileContext(nc) as tc, ExitStack() as ctx:
        wpool = ctx.enter_context(tc.tile_pool(name="wts", bufs=1))
        spool = ctx.enter_context(tc.tile_pool(name="small", bufs=1))
        nbufs = 1 if any(flags) else 3
        obufs = 1 if any(flags) else 4
        hpool = ctx.enter_context(tc.tile_pool(name="h", bufs=nbufs))
        opool = ctx.enter_context(tc.tile_pool(name="o", bufs=obufs))
        pwide = ctx.enter_context(tc.tile_pool(name="pw", bufs=2, space="PSUM"))
        psmall = ctx.enter_context(tc.tile_pool(name="ps", bufs=2, space="PSUM"))

        # ---------------- DMAs (issue order ~ arrival order) ----------------
        xnb_t = spool.tile([NNB, DIM], F32, tag="xnb")
        nc.sync.dma_start(xnb_t[:], xnb_d[:])
        nmp_t = spool.tile([NNB, 1], F32, tag="nmp")
        nc.sync.dma_start(nmp_t[:], nm_d.unsqueeze(1))
        nmr_t = spool.tile([1, NNB], F32, tag="nmr")
        nc.sync.dma_start(nmr_t[:], nm_d.unsqueeze(0))
        i128_t = spool.tile([128, 128], BF16, tag="i128")
        nc.sync.dma_start(i128_t[:], i128_d[:])

        def row_tile(d_ap, tag):
            t = spool.tile([1, DIM], F32, tag=tag)
            nc.sync.dma_start(t[:], d_ap.unsqueeze(0))
            return t

        qb_t = row_tile(qb_d, "qbr") if use_qb else None
        ob_t = row_tile(ob_d, "obr") if use_ob else None
        if use_vb:
            vbt_t = spool.tile([128, NJ], BF16, tag="vbt")
            nc.sync.dma_start(vbt_t[:], vb_d[:])
        g_t = row_tile(g_d, "gr") if use_g else None
        b_t = row_tile(b_d, "br") if use_b else None
        if use_kb:
            kbt_t = spool.tile([128, NJ], BF16, tag="kbt")
            nc.sync.dma_start(kbt_t[:], kbt_d[:])
        if use_mask:
            mrow_t = spool.tile([1, SEQ], I32, tag="mrow")
            nc.sync.dma_start(mrow_t[:], mask_d.unsqueeze(0))

        def load_mat(d_ap, dt, tagp):
            ts = []
            for j in range(NJ):
                t = wpool.tile([128, d_ap.shape[1]], dt, tag=f"{tagp}{j}")
                nc.sync.dma_start(t[:], d_ap[j * 128:(j + 1) * 128, :])
                ts.append(t)
            return ts

        qwt_t = load_mat(qwt_d, BF16, "qwt")
        kw_t = load_mat(kw_d, BF16, "kw")
        xt_t = load_mat(xt_d, BF16, "xt")
        xb_t = load_mat(xbn_d, BF16, "xb")
        x_t = load_mat(x_d, F32, "x")
        vwt_t = load_mat(vwt_d, BF16, "vwt")
        owt_t = load_mat(owt_d, BF16, "owt")

        ones11 = spool.tile([1, 1], BF16, tag="ones11")
        nc.vector.memset(ones11[:], 1.0)
        ones1x128 = spool.tile([1, 128], F32, tag="ones1x128")
        nc.vector.memset(ones1x128[:], 1.0)

        # touch every ACT function family once so the ~1.3us table loads
        # happen during the DMA fill instead of on the critical tail
        dummy_t = spool.tile([1, 1], F32, tag="dummy")
        nc.vector.memset(dummy_t[:], 1.0)
        for fn in (AF.Exp, AF.Identity, AF.Sqrt, AF.Square, AF.Copy):
            nc.scalar.activation(dummy_t[:], dummy_t[:], fn)

        def bcast_row(row_ap, out_tile, nrows):
            """out[p, :] = row[0, :] for p in range(nrows), via PE rank-1."""
            n = out_tile.shape[-1]
            pb = pwide.tile([128, DIM], F32, tag="wide")
            for h0 in range(0, n, 512):
                hi = min(h0 + 512, n)
                nc.tensor.matmul(pb[:nrows, h0:hi], lhsT=ones1x128[0:1, 0:nrows],
                                 rhs=row_ap[0:1, h0:hi], start=True, stop=True)
            nc.scalar.copy(out_tile[:nrows, :], pb[:nrows, 0:n])

        # ---------------- neighbor pooling: SxnT (bf16 [128, NJ]) ----------
        sxnt_t = spool.tile([128, NJ], BF16, tag="sxnt")
        for j in range(NJ):
            ps = psmall.tile([128, 16], F32, tag="psm")
            nc.tensor.matmul(ps[:, 0:1], lhsT=xnb_t[:, j * 128:(j + 1) * 128],
                             rhs=nmp_t[:], start=True, stop=True)
            nc.scalar.copy(sxnt_t[:, j:j + 1], ps[:, 0:1])
        cnt_t = spool.tile([1, 1], F32, tag="cnt")
        nc.vector.reduce_sum(cnt_t[:], nmr_t[:], AX.X)
        rcnt_t = spool.tile([1, 1], F32, tag="rcnt")
        nc.vector.reciprocal(rcnt_t[:], cnt_t[:])

        # ---------------- qvec = (qw @ xn + qb) / 8  (fp32 [1, DIM]) -------
        pqv = pwide.tile([128, DIM], F32, tag="wide")
        for j in range(NJ):
            for h0 in (0, 512):
                nc.tensor.matmul(pqv[0:1, h0:h0 + 512], lhsT=sxnt_t[:, j:j + 1],
                                 rhs=qwt_t[j][:, h0:h0 + 512],
                                 start=(j == 0), stop=(j == NJ - 1))
        qvec_t = spool.tile([1, DIM], F32, tag="qvec")
        nc.vector.tensor_scalar(qvec_t[:], pqv[0:1, :], rcnt_t[:], 0.125,
                                AluOpType.mult, AluOpType.mult)
        if use_qb:
            qb8_t = spool.tile([1, DIM], F32, tag="qb8")
            nc.vector.tensor_scalar_mul(qb8_t[:], qb_t[:], 0.125)
            nc.vector.tensor_tensor(qvec_t[:], qvec_t[:], qb8_t[:], op=AluOpType.add)

        # ---------------- per-chunk head-blocked qvec (bf16 [128, NH]) -----
        qvr_t = spool.tile([1, DIM], BF16, tag="rowb")
        nc.vector.tensor_copy(qvr_t[:], qvec_t[:])
        blk_t = []
        for j in range(NJ):
            bt = spool.tile([128, NH], BF16, tag=f"blk{j}")
            nc.vector.memset(bt[:], 0.0)
            pt = psmall.tile([128, 16], BF16, tag="psmb")
            nc.tensor.transpose(pt[:, 0:1], qvr_t[0:1, j * 128:(j + 1) * 128],
                                ones11[:])
            nc.vector.tensor_copy(bt[0:64, 2 * j:2 * j + 1], pt[0:64, 0:1])
            nc.vector.tensor_copy(bt[64:128, 2 * j + 1:2 * j + 2], pt[64:128, 0:1])
            blk_t.append(bt)

        # ---------------- qk[h, c] = sum_d q[h, d] kw[64h+d, c] ------------
        pqk = pwide.tile([128, DIM], F32, tag="wide")
        for j in range(NJ):
            for h0 in (0, 512):
                nc.tensor.matmul(pqk[0:NH, h0:h0 + 512], lhsT=blk_t[j][:],
                                 rhs=kw_t[j][:, h0:h0 + 512],
                                 start=(j == 0), stop=(j == NJ - 1))
        qk_t = spool.tile([NH, DIM], BF16, tag="qk")
        nc.scalar.copy(qk_t[:], pqk[0:NH, :])
        if use_kb:
            pqkb = psmall.tile([128, 16], F32, tag="psm")
            for j in range(NJ):
                nc.tensor.matmul(pqkb[0:NH, 0:1], lhsT=blk_t[j][:],
                                 rhs=kbt_t[:, j:j + 1],
                                 start=(j == 0), stop=(j == NJ - 1))
            qkb_t = spool.tile([NH, 1], F32, tag="qkb")
            nc.vector.tensor_copy(qkb_t[:], pqkb[0:NH, 0:1])

        # ---------------- scoresT [NH, SEQ] = qk @ x^T ---------------------
        qkt_t = []
        for j in range(NJ):
            t = spool.tile([128, NH], BF16, tag=f"qkt{j}")
            pt = psmall.tile([128, 16], BF16, tag="psmb")
            nc.tensor.transpose(pt[:], qk_t[:, j * 128:(j + 1) * 128],
                                i128_t[0:NH, 0:NH])
            nc.scalar.copy(t[:], pt[:])
            qkt_t.append(t)
        psc = pwide.tile([128, DIM], F32, tag="wide")
        for j in range(NJ):
            for h0 in (0, 512):
                nc.tensor.matmul(psc[0:NH, h0:h0 + 512], lhsT=qkt_t[j][:],
                                 rhs=xt_t[j][:, h0:h0 + 512],
                                 start=(j == 0), stop=(j == NJ - 1))

        # ---------------- softmax over seq (keys) --------------------------
        # scores are O(1) here (q is a pooled mean), so exp without
        # max-subtraction is safe; masked keys multiply to exactly 0.
        w_t = spool.tile([NH, SEQ], BF16, tag="w")
        den_t = spool.tile([NH, 1], F32, tag="den")
        expbias = qkb_t[:] if use_kb else 0.0
        if not use_mask:
            nc.scalar.activation(w_t[:], psc[0:NH, :], AF.Exp, bias=expbias,
                                 scale=1.0, accum_out=den_t[:])
        else:
            nc.scalar.activation(w_t[:], psc[0:NH, :], AF.Exp, bias=expbias,
                                 scale=1.0)
            mrowf_t = spool.tile([1, SEQ], F32, tag="mrowf")
            nc.vector.tensor_copy(mrowf_t[:], mrow_t[:])
            ind_t = spool.tile([1, SEQ], F32, tag="ind")
            nc.vector.tensor_scalar(ind_t[:], mrowf_t[:], 0.0, None,
                                    AluOpType.not_equal)
            m16_t = spool.tile([NH, SEQ], F32, tag="bvb")
            bcast_row(ind_t, m16_t, NH)
            nc.vector.scalar_tensor_tensor(w_t[:], w_t[:], 1.0, m16_t[:],
                                           AluOpType.mult, AluOpType.mult,
                                           accum_out=den_t[:])
        rden_t = spool.tile([NH, 1], F32, tag="rden")
        nc.vector.reciprocal(rden_t[:], den_t[:])

        # -------- early LN stats: per-row mean/var of x (overlaps DMA) -----
        mvx_t = []
        for t in range(NT):
            xv = x_t[t][:].rearrange("p (g f) -> p g f", g=2)
            st_t = hpool.tile([128, 2, 6], F32, tag="st")
            nc.vector.bn_stats(st_t[:, 0, :], xv[:, 0, :])
            nc.vector.bn_stats(st_t[:, 1, :], xv[:, 1, :])
            mv = spool.tile([128, 2], F32, tag=f"mvx{t}")
            nc.vector.bn_aggr(mv[:], st_t[:])
            mvx_t.append(mv)

        # ---------------- pooled[h, c] = sum_s w[h, s] x[s, c] -------------
        wt_t = []
        for j in range(NT):
            t = spool.tile([128, NH], BF16, tag=f"wt{j}")
            pt = psmall.tile([128, 16], BF16, tag="psmb")
            nc.tensor.transpose(pt[:], w_t[:, j * 128:(j + 1) * 128],
                                i128_t[0:NH, 0:NH])
            nc.vector.tensor_copy(t[:], pt[:])
            wt_t.append(t)
        ppl = pwide.tile([128, DIM], F32, tag="wide")
        for j in range(NT):
            for h0 in (0, 512):
                nc.tensor.matmul(ppl[0:NH, h0:h0 + 512], lhsT=wt_t[j][:],
                                 rhs=xb_t[j][:, h0:h0 + 512],
                                 start=(j == 0), stop=(j == NT - 1))
        pn_t = spool.tile([NH, DIM], BF16, tag="pn")
        nc.vector.tensor_scalar_mul(pn_t[:], ppl[0:NH, :], rden_t[:])

        # ---------------- context: diag blocks of pn @ vw^T ----------------
        pnt_t = []
        for j in range(NJ):
            t = spool.tile([128, NH], BF16, tag=f"pnt{j}")
            pt = psmall.tile([128, 16], BF16, tag="psmb")
            nc.tensor.transpose(pt[:], pn_t[:, j * 128:(j + 1) * 128],
                                i128_t[0:NH, 0:NH])
            nc.scalar.copy(t[:], pt[:])
            pnt_t.append(t)
        pcx = pwide.tile([128, DIM], F32, tag="wide")
        for j in range(NJ):
            for h0 in (0, 512):
                nc.tensor.matmul(pcx[0:NH, h0:h0 + 512], lhsT=pnt_t[j][:],
                                 rhs=vwt_t[j][:, h0:h0 + 512],
                                 start=(j == 0), stop=(j == NJ - 1))
        # ctx[o] = pcx[head(o), o]: copy to SBUF, transpose 128-col slices,
        # then pick the two half-column blocks (32-aligned partition bases).
        pcs_t = spool.tile([NH, DIM], BF16, tag="pcs")
        nc.scalar.copy(pcs_t[:], pcx[0:NH, :])
        cxt_t = spool.tile([128, NJ], BF16, tag="cxt")
        for j in range(NJ):
            pt = psmall.tile([128, 16], BF16, tag="psmb")
            nc.tensor.transpose(pt[:], pcs_t[:, j * 128:(j + 1) * 128],
                                i128_t[0:NH, 0:NH])
            nc.vector.tensor_copy(cxt_t[0:64, j:j + 1], pt[0:64, 2 * j:2 * j + 1])
            nc.vector.tensor_copy(cxt_t[64:128, j:j + 1],
                                  pt[64:128, 2 * j + 1:2 * j + 2])
        if use_vb:
            nc.vector.tensor_tensor(cxt_t[:], cxt_t[:], vbt_t[:], op=AluOpType.add)

        # ---------------- out_vec = ow @ ctx + ob --------------------------
        pov = pwide.tile([128, DIM], F32, tag="wide")
        for j in range(NJ):
            for h0 in (0, 512):
                nc.tensor.matmul(pov[0:1, h0:h0 + 512], lhsT=cxt_t[:, j:j + 1],
                                 rhs=owt_t[j][:, h0:h0 + 512],
                                 start=(j == 0), stop=(j == NJ - 1))
        bvec_t = spool.tile([1, DIM], F32, tag="bvec")
        nc.scalar.copy(bvec_t[:], pov[0:1, :])
        if use_ob:
            nc.vector.tensor_tensor(bvec_t[:], bvec_t[:], ob_t[:], op=AluOpType.add)

        # ---------------- residual + LayerNorm -----------------------------
        # h = x + v (v = bvec broadcast over rows).  Per row s:
        #   mu_h[s]  = mean_x[s] + mu_v
        #   var_h[s] = var_x[s] + var_v + 2*(Sxv[s]/D - mean_x[s]*mu_v)
        # so only the cheap cross-term Sxv = x @ v (PE gemv on xT) and
        # per-tile scalar fixups happen after bvec is known.
        bvb_t = spool.tile([128, DIM], F32, tag="bvb")
        nc.gpsimd.partition_broadcast(bvb_t[:], bvec_t[:])
        if use_g:
            gb_t = spool.tile([128, DIM], F32, tag="gb")
            bcast_row(g_t, gb_t, 128)
        if use_b:
            bb_t = spool.tile([128, DIM], F32, tag="bb")
            bcast_row(b_t, bb_t, 128)

        # scalars of v: sv = [mu_v, var_v]
        sv_t = spool.tile([1, 2], F32, tag="sv")
        nc.vector.reduce_sum(sv_t[0:1, 0:1], bvec_t[:], AX.X)
        junk_t = spool.tile([1, DIM], F32, tag="qvec")
        nc.scalar.activation(junk_t[:], bvec_t[:], AF.Square,
                             accum_out=sv_t[0:1, 1:2])
        nc.vector.tensor_scalar_mul(sv_t[:], sv_t[:], 1.0 / DIM)
        muv2_t = spool.tile([1, 1], F32, tag="muv2")
        nc.vector.tensor_tensor(muv2_t[:], sv_t[0:1, 0:1], sv_t[0:1, 0:1],
                                op=AluOpType.mult)
        nc.vector.tensor_tensor(sv_t[0:1, 1:2], sv_t[0:1, 1:2], muv2_t[:],
                                op=AluOpType.subtract)
        bsc_t = spool.tile([128, 2], F32, tag="bsc")
        nc.gpsimd.partition_broadcast(bsc_t[:], sv_t[:])

        # Sxv row via PE: bvecT chunks (bf16) against xT
        bvr_t = spool.tile([1, DIM], BF16, tag="rowb")
        nc.vector.tensor_copy(bvr_t[:], bvec_t[:])
        bvt_t = spool.tile([128, NJ], BF16, tag="bvt")
        for j in range(NJ):
            pt = psmall.tile([128, 16], BF16, tag="psmb")
            nc.tensor.transpose(pt[:, 0:1], bvr_t[0:1, j * 128:(j + 1) * 128],
                                ones11[:])
            nc.vector.tensor_copy(bvt_t[:, j:j + 1], pt[:, 0:1])
        psxv = pwide.tile([128, DIM], F32, tag="wide")
        for j in range(NJ):
            for h0 in (0, 512):
                nc.tensor.matmul(psxv[0:1, h0:h0 + 512], lhsT=bvt_t[:, j:j + 1],
                                 rhs=xt_t[j][:, h0:h0 + 512],
                                 start=(j == 0), stop=(j == NJ - 1))
        sxvr_t = spool.tile([1, SEQ], BF16, tag="rowb")
        nc.scalar.copy(sxvr_t[:], psxv[0:1, :])
        sxvc_t = spool.tile([128, NT], F32, tag="sxvc")
        for t in range(NT):
            pt = psmall.tile([128, 16], BF16, tag="psmb")
            nc.tensor.transpose(pt[:, 0:1], sxvr_t[0:1, t * 128:(t + 1) * 128],
                                ones11[:])
            nc.vector.tensor_copy(sxvc_t[:, t:t + 1], pt[:, 0:1])

        for t in range(NT):
            mvx = mvx_t[t]
            a_t = hpool.tile([128, 1], F32, tag="a")
            nc.vector.tensor_scalar_mul(a_t[:], sxvc_t[:, t:t + 1], 2.0 / DIM)
            b_t2 = hpool.tile([128, 1], F32, tag="b2")
            nc.vector.tensor_tensor(b_t2[:], mvx[:, 0:1], bsc_t[:, 0:1],
                                    op=AluOpType.mult)
            c_t = hpool.tile([128, 1], F32, tag="c")
            nc.vector.scalar_tensor_tensor(c_t[:], b_t2[:], -2.0, a_t[:],
                                           AluOpType.mult, AluOpType.add)
            d_t = hpool.tile([128, 1], F32, tag="d")
            nc.vector.tensor_scalar(d_t[:], c_t[:], bsc_t[:, 1:2], LN_EPS,
                                    AluOpType.add, AluOpType.add)
            e_t = hpool.tile([128, 1], F32, tag="e")
            nc.vector.tensor_tensor(e_t[:], d_t[:], mvx[:, 1:2],
                                    op=AluOpType.add)
            rv_t = hpool.tile([128, 1], F32, tag="rv")
            nc.vector.reciprocal(rv_t[:], e_t[:])
            rstd_t = hpool.tile([128, 1], F32, tag="rstd")
            nc.scalar.sqrt(rstd_t[:], rv_t[:])
            muh_t = hpool.tile([128, 1], F32, tag="muh")
            nc.vector.tensor_tensor(muh_t[:], mvx[:, 0:1], bsc_t[:, 0:1],
                                    op=AluOpType.add)
            nmr_t = hpool.tile([128, 1], F32, tag="nmr")
            nc.vector.scalar_tensor_tensor(nmr_t[:], muh_t[:], -1.0, rstd_t[:],
                                           AluOpType.mult, AluOpType.mult)
            t1_t = hpool.tile([128, DIM], F32, tag="h")
            eng = nc.vector if t % 2 == 0 else nc.gpsimd
            eng.tensor_tensor(t1_t[:], x_t[t][:], bvb_t[:], op=AluOpType.add)
            o_t = opool.tile([128, DIM], F32, tag="o")
            nc.scalar.activation(o_t[:], t1_t[:], AF.Identity, bias=nmr_t[:],
                                 scale=rstd_t[:])
            if use_g:
                nc.vector.tensor_tensor(o_t[:], o_t[:], gb_t[:], op=AluOpType.mult)
            if use_b:
                nc.vector.tensor_tensor(o_t[:], o_t[:], bb_t[:], op=AluOpType.add)
            nc.sync.dma_start(out_d[t * 128:(t + 1) * 128, :], o_t[:])

    nc.compile()
    return nc


def _get_program(flags):
    if flags not in _cache:
        _cache[flags] = _build(flags)
    return _cache[flags]


def kernel(**inputs):
    f32 = lambda a: np.ascontiguousarray(np.asarray(a, np.float32))
    bf = ml_dtypes.bfloat16
    x = f32(inputs["x"])
    xnb = f32(inputs["x_neighbor"])
    mask = np.ascontiguousarray(np.asarray(inputs["mask"], np.int32))
    nmask = f32(inputs["neighbor_mask"])
    qw, qb = f32(inputs["qw"]), f32(inputs["qb"])
    kw, kb = f32(inputs["kw"]), f32(inputs["kb"])
    vw, vb = f32(inputs["vw"]), f32(inputs["vb"])
    ow, ob = f32(inputs["ow"]), f32(inputs["ob"])
    ln_g, ln_b = f32(inputs["ln_g"]), f32(inputs["ln_b"])

    flags = (bool(qb.any()), bool(kb.any()), bool(vb.any()), bool(ob.any()),
             bool((ln_g != 1.0).any()), bool(ln_b.any()), bool((mask == 0).any()))
    nc = _get_program(flags)
    use_qb, use_kb, use_vb, use_ob, use_g, use_b, use_mask = flags

    qwt = np.ascontiguousarray(qw.T).astype(bf)
    kwb = kw.astype(bf)
    vwt = np.ascontiguousarray(vw.T).astype(bf)
    owt = np.ascontiguousarray(ow.T).astype(bf)
    i128 = np.eye(128, dtype=ml_dtypes.bfloat16)

    in_maps = []
    for b in range(BS):
        m = {
            "x": np.ascontiguousarray(x[b]),
            "xt": np.ascontiguousarray(x[b].T).astype(bf),
            "xbn": x[b].astype(bf),
            "qwt": qwt, "kw": kwb, "vwt": vwt, "owt": owt,
            "xnb": np.ascontiguousarray(xnb[b]),
            "nm": np.ascontiguousarray(nmask[b]),
            "i128": i128,
        }
        if use_qb:
            m["qb"] = qb
        if use_kb:
            m["kbt"] = np.ascontiguousarray(kb.reshape(NJ, 128).T).astype(bf)
        if use_vb:
            m["vbt"] = np.ascontiguousarray(vb.reshape(NJ, 128).T).astype(bf)
        if use_ob:
            m["ob"] = ob
        if use_g:
            m["lng"] = ln_g
        if use_b:
            m["lnb"] = ln_b
        if use_mask:
            m["mask"] = np.ascontiguousarray(mask[b])
        in_maps.append(m)

    res = bass_utils.run_bass_kernel_spmd(nc, in_maps, core_ids=list(range(N_CORES)))
    return np.stack([res.results[b]["out"] for b in range(BS)]).astype(np.float32)

